# revision 14
# baseline (speedup 1.0000x reference)
"""Trainium2 Bass kernel for nn_BaseMultiHeadAttention (B=2, S=2048, E=1024, H=16).

Sharding: core = (batch, head-group of 4). Each of the 8 NeuronCores runs the
full attention pipeline for 4 heads of one batch element, computes a partial
output projection over its 256 context features, and writes a [S, E] fp16
partial; the host sums the 4 partials per batch (the all-reduce) and adds the
bias.

Device pipeline per core (all-fp16 compute, fp32 PSUM accumulate):
  Phase A (per head, per 8-tile half): one gpsimd casting DMA loads the
    host-interleaved [q|k] block fp32->fp16; one fused square + one grouped
    reduce give both RMS sums; sqrt (ACT) + reciprocal (DVE). q is
    normalized on DVE; k's scale is folded into the exp scale AP (per-PSUM-
    partition = per k position), so k skips the normalize multiply. RoPE
    uses host-doubled cos/sin tables: 2 full-width muls + add/sub (fp16 2x
    DVE) write q into cols [0:64) and k into cols [64:128) of a padded
    staging slot; one XBAR DMA-transpose yields qkT[128, t, s] (q_d on
    partitions 0-63) and a second, 64-col-shifted one yields kT - no PE
    transposes, no PSUM->SBUF copies.
  Attention (per 1024-q chunk-pair P, per head) split into sc() and ct():
    sc: per k-tile jj, matmuls fill scoresT [128 k, 1024 q] PSUM causal-
    tight (only the q >= jj*128 suffix), exp on ACT (scale AP = D^-0.5 *
    rms_k) into fp16 p tiles, gpsimd affine_select zeroes the diagonal
    triangle. ct: ctx = p.T@[v|1] accumulates in PSUM (ones column = softmax
    row sums), rows scaled by 1/sum on the PSUM read into cpair. P0 needs
    only phase-A half 0, so the exp stream starts early; P0's last heads run
    at the end so the drain tail is the small chunk. Emission order is hand-
    pipelined per engine (streams are in-order).
  Projection (per 128-row tile): XBAR DMA-transpose cpair -> ctxT [128,2,128]
    prefetched right after the last head's normalize, 2x2 matmuls against
    fp16 wt, PSUM->SBUF fp16 copies (DVE/Pool alternating), fp16 DMA out.
"""
import numpy as np

import bass_rust
import concourse.bass as bass
import concourse.mybir as mybir
import concourse.tile as tile
from concourse.bass_utils import run_bass_kernel_spmd

B, S, E, H, D = 2, 2048, 1024, 16, 64
HD = D // 2
N_CORES = 8
HL = 4                     # heads per core
NJ = HL                    # jobs per core = heads (single batch per core)
NT = S // 128              # 16 s-tiles
NH = NT // 2               # tiles per phase-A half
EPS = 1.1920928955078125e-07
f32 = mybir.dt.float32
f16 = mybir.dt.float16
ALU = mybir.AluOpType
ACTF = mybir.ActivationFunctionType
AXX = mybir.AxisListType.X

_TC = tile.TileContext


def _legalize_waits(nc):
    """Split multi-wait sync_infos for this walrus build.

    This neuronxcc's codegen allows 1 sync wait per instruction (2 on
    EventSemaphore), while the Tile scheduler attaches all outstanding
    waits to one instruction.  Hoist the excess onto same-engine NoOps
    inserted immediately before the offending instruction - the engine
    executes its stream in order, so blocking semantics are identical.
    """
    uid = 0
    for f in nc.m.functions:
        for blk in f.blocks:
            insts = list(blk.instructions)
            out, changed = [], False
            for inst in insts:
                si = inst.sync_info
                cap = 2 if isinstance(inst, mybir.InstEventSemaphore) else 1
                if si is not None and len(si.on_wait) > cap:
                    changed = True
                    waits = list(si.on_wait)
                    for w in waits[:-cap]:
                        carrier = mybir.InstNoOp(
                            name=f"legwait-{uid}", engine=inst.engine,
                            ins=[], outs=[])
                        uid += 1
                        carrier.sync_info = bass_rust.SyncInfo(
                            on_wait=[w], on_update=[])
                        nc.register_instruction(carrier, overwrite=True)
                        out.append(carrier)
                    si.on_wait = waits[-cap:]
                    inst.sync_info = si
                out.append(inst)
            if changed:
                blk.instructions = out


def _flat_cols(src, start, count):
    """AP over a [128, ...] slice viewed as flat columns [start, start+count)."""
    return bass.AP(tensor=src.tensor, offset=src.offset + start,
                   ap=[list(src.ap[0]), [1, count]])


def build_nc():
    nc = bass.Bass("TRN2", target_bir_lowering=False, debug=False)
    qk_in = nc.dram_tensor("qk", [NJ, 128, NT, 128], f32, kind="ExternalInput")
    v_in = nc.dram_tensor("v", [NJ, 128, NT, D + 1], f32, kind="ExternalInput")
    cs_in = nc.dram_tensor("cs", [128, 2, NT, D], f32, kind="ExternalInput")
    wt_in = nc.dram_tensor("wt", [128, 2, E], f32, kind="ExternalInput")
    out = nc.dram_tensor("out", [NT, 128, E], f16, kind="ExternalOutput")

    with _TC(nc) as tc, nc.allow_low_precision(reason="fp16 attention"):
        with tc.tile_pool(name="const", bufs=1) as cp, \
             tc.tile_pool(name="pl", bufs=5) as pl, \
             tc.tile_pool(name="pa", bufs=3) as pa, \
             tc.tile_pool(name="pp", bufs=34) as pp, \
             tc.tile_pool(name="pb", bufs=4) as pb, \
             tc.tile_pool(name="pt", bufs=10) as ptp, \
             tc.tile_pool(name="po", bufs=4) as po, \
             tc.tile_pool(name="cpr", bufs=2) as cpr, \
             tc.tile_pool(name="ps_s", bufs=2, space="PSUM") as ps_s, \
             tc.tile_pool(name="ps_c", bufs=2, space="PSUM") as ps_c, \
             tc.tile_pool(name="ps_o", bufs=2, space="PSUM") as ps_o:
            eps_t = cp.tile([128, 1], f32)
            nc.vector.memset(eps_t, EPS)
            eps64_t = cp.tile([128, 1], f32)
            nc.vector.memset(eps64_t, D * EPS)
            cs_sb = cp.tile([128, 2, NT, D], f16)   # doubled [cos|cos],[sin|sin]
            wt_sb = cp.tile([128, 2, E], f16)
            vsb = cp.tile([128, NJ, NT, D + 1], f16)
            qkin = cp.tile([128, 2, NT + 1, 128], f16)   # 2-slot ring by job
            qkT = cp.tile([128, NJ, NT, 128], f16)
            kT = cp.tile([128, NJ, NT, 128], f16)
            rs2 = cp.tile([128, NJ, NT, 2], f32)    # interleaved rs_q, rs_k

            for sl_ in range(2):
                # junk pad block read by the shifted k transpose
                nc.gpsimd.memset(qkin[:, sl_, NT, :], 0.0)

            # ---------------- Phase A: norm + rope + transposes ------------
            def load_qk(j, h=None):
                ts = slice(0, NT) if h is None else slice(h * NH, (h + 1) * NH)
                n = ts.stop - ts.start
                qkr = pl.tile([128, n, 2, D], f16, tag="qkr", name="qkr")
                nc.gpsimd.dma_start(out=qkr, in_=qk_in.ap()[j][:, ts])
                return qkr

            def phase_a(j, h, qkr, hofs=0):
                ts = slice(h * NH, (h + 1) * NH)
                to = slice(hofs * NH, hofs * NH + NH)
                qr = qkr[:, to, 0, :]
                kr = qkr[:, to, 1, :]
                sq = pa.tile([128, NH, 2, D], f16, tag="sq", name="sq")
                nc.vector.tensor_mul(sq, qkr[:, to], qkr[:, to])
                rs_sl = rs2[:, j, ts, :]
                nc.vector.reduce_sum(rs_sl, sq, axis=AXX)
                # rs_q = 1/sqrt(ss/D + eps); rs_k = D^-0.5/sqrt(ss/D + eps)
                # = 1/sqrt(ss + D*eps), folded into the exp scale AP
                nc.scalar.activation(
                    out=rs2[:, j, ts, 0], in_=rs2[:, j, ts, 0],
                    func=ACTF.Sqrt, bias=eps_t, scale=1.0 / D)
                nc.scalar.activation(
                    out=rs2[:, j, ts, 1], in_=rs2[:, j, ts, 1],
                    func=ACTF.Sqrt, bias=eps64_t, scale=1.0)
                nc.vector.reciprocal(out=rs_sl, in_=rs_sl)
                xq = pa.tile([128, NH, D], f16, tag="xq", name="xq")
                rsb = bass.AP(
                    tensor=rs2.tensor,
                    offset=rs2.offset + (j * NT + ts.start) * 2,
                    ap=[list(rs2.ap[0]), [2, NH], [0, D]])
                nc.vector.tensor_mul(xq, qr, rsb)
                csl, ssl = cs_sb[:, 0, ts, :], cs_sb[:, 1, ts, :]
                sl_ = j % 2
                for src, off in ((xq, 0), (kr, 64)):
                    # src cols = [x1|x2]; doubled tables: tc=[x1c|x2c],
                    # tsn=[x1s|x2s]; o1 = x1c - x2s; o2 = x1s + x2c
                    tc = pa.tile([128, NH, D], f16, tag="tc", name="tc")
                    tsn = pa.tile([128, NH, D], f16, tag="tsn", name="tsn")
                    nc.vector.tensor_mul(tc, src, csl)
                    nc.vector.tensor_mul(tsn, src, ssl)
                    nc.vector.tensor_sub(
                        qkin[:, sl_, ts, off:off + HD],
                        tc[:, :, 0:HD], tsn[:, :, HD:D])
                    nc.vector.tensor_add(
                        qkin[:, sl_, ts, off + HD:off + D],
                        tsn[:, :, 0:HD], tc[:, :, HD:D])
                src2d = qkin[:, sl_, 0, 0:1]
                nc.sync.dma_start_transpose(
                    out=qkT[:, j, ts, :],
                    in_=_flat_cols(src2d, ts.start * 128, NH * 128))
                nc.sync.dma_start_transpose(
                    out=kT[:, j, ts, :],
                    in_=_flat_cols(src2d, ts.start * 128 + 64, NH * 128))

            # ---------------- Attention scores+exp for chunk-pair P --------
            pts_map = {}

            def sc(P, j):
                njj = 8 * P + 8
                pts = []
                for jj in range(njj):
                    lo = max(jj * 128 - 1024 * P, 0)
                    sps = ps_s.tile([128, 1024], f32, tag="s", name="sps")
                    pieces = [(lo, 512), (512, 1024)] if lo < 512 \
                        else [(lo, 1024)]
                    for a, b in pieces:
                        nc.tensor.matmul(
                            sps[:, a:b],
                            lhsT=kT[0:64, j, jj, :],
                            rhs=qkT[0:64, j,
                                    8 * P + a // 128:8 * P + b // 128, :],
                            start=True, stop=True)
                    pt = pp.tile([128, 1024], f16, tag="p", name="pt")
                    nc.scalar.activation(
                        out=pt[:, lo:1024], in_=sps[:, lo:1024],
                        func=ACTF.Exp, scale=rs2[:, j, jj, 1:2])
                    if jj >= 8 * P:
                        psl = pt[:, lo:lo + 128]
                        nc.gpsimd.affine_select(
                            out=psl, in_=psl, compare_op=ALU.is_ge,
                            fill=0.0, base=0, pattern=[[1, 128]],
                            channel_multiplier=-1)
                    pts.append(pt)
                pts_map[(P, j)] = pts

            # ---------------- ctx + normalize (+ ctxT prefetch) ------------
            ctxT_tiles = {}

            def ct(P, j, cp_t, last_head=False, halves=(0, 1)):
                pts = pts_map[(P, j)]
                for half in halves:
                    ctxp = ps_c.tile([128, 4, D + 1], f32, tag="c",
                                     name="ctxp")
                    for qb in range(4):
                        i = 8 * P + 4 * half + qb
                        ic = (4 * half + qb) * 128
                        for jj in range(i + 1):
                            nc.tensor.matmul(
                                ctxp[:, qb, :],
                                lhsT=pts[jj][:, ic:ic + 128],
                                rhs=vsb[:, j, jj, :],
                                start=(jj == 0), stop=(jj == i))
                    if not last_head:
                        rsum = pb.tile([128, 4], f32, tag="rsum",
                                       name="rsum")
                        nc.vector.reciprocal(out=rsum, in_=ctxp[:, :, D])
                        rsum_b = bass.AP(
                            tensor=rsum.tensor, offset=rsum.offset,
                            ap=[list(rsum.ap[0]), [1, 4], [0, D]])
                        nc.vector.tensor_mul(
                            cp_t[:, 4 * half:4 * half + 4,
                                 64 * j:64 * j + 64],
                            ctxp[:, :, 0:D], rsum_b)
                    else:
                        # last head: per-2-tile normalize + transpose so the
                        # proj chain starts before the whole half is done
                        ctxT4 = ptp.tile([128, 8, 128], f16, tag="ctxT",
                                         name="ctxT4")
                        for pr in range(2):
                            t2 = slice(4 * half + 2 * pr,
                                       4 * half + 2 * pr + 2)
                            rsum = pb.tile([128, 2], f32, tag="rsum",
                                           name="rsum")
                            nc.vector.reciprocal(
                                out=rsum, in_=ctxp[:, 2 * pr:2 * pr + 2, D])
                            rsum_b = bass.AP(
                                tensor=rsum.tensor, offset=rsum.offset,
                                ap=[list(rsum.ap[0]), [1, 2], [0, D]])
                            nc.vector.tensor_mul(
                                cp_t[:, t2, 64 * j:64 * j + 64],
                                ctxp[:, 2 * pr:2 * pr + 2, 0:D], rsum_b)
                            csrc = cp_t[:, t2.start, 0:1]
                            nc.sync.dma_start_transpose(
                                out=ctxT4[:, 4 * pr:4 * pr + 4, :],
                                in_=_flat_cols(csrc, 0, 512))
                        ctxT_tiles[(P, half)] = ctxT4
                if last_head and halves[-1] == 1:
                    del pts_map[(P, j)]

            # ---------------- Projection for s-tiles of chunk-pair P -------
            def proj(P, half, mixed=False):
                ctxT4 = ctxT_tiles.pop((P, half))
                for pair in range(2):
                    osb = po.tile([128, 2, E], f16, tag="osb", name="osb")
                    for tp in range(2):
                        ip = 2 * pair + tp
                        for eh in range(2):
                            pro = ps_o.tile([128, 512], f32, tag="o",
                                            name="pro")
                            nc.tensor.matmul(
                                pro, lhsT=ctxT4[:, 2 * ip, :],
                                rhs=wt_sb[:, 0, eh * 512:(eh + 1) * 512],
                                start=True, stop=False)
                            nc.tensor.matmul(
                                pro, lhsT=ctxT4[:, 2 * ip + 1, :],
                                rhs=wt_sb[:, 1, eh * 512:(eh + 1) * 512],
                                start=False, stop=True)
                            dst = osb[:, tp, eh * 512:(eh + 1) * 512]
                            if mixed and (tp + eh) % 2 == 0:
                                nc.scalar.copy(dst, pro)
                            else:
                                nc.vector.tensor_copy(dst, pro)
                    i0 = 8 * P + 4 * half + 2 * pair
                    nc.sync.dma_start(
                        out=out.ap()[i0:i0 + 2].rearrange("t p e -> p t e"),
                        in_=osb)

            # ---------------- Emission schedule ----------------------------
            # gpsimd casts first (Pool is in-order; selects would starve
            # them); compute emission is hand-pipelined so that no engine's
            # in-order stream blocks work another engine needs soon.
            qkr00 = load_qk(0, 0)
            nc.gpsimd.dma_start(out=cs_sb, in_=cs_in.ap())
            qkr01 = load_qk(0, 1)
            qkr1 = load_qk(1)
            nc.gpsimd.dma_start(out=vsb[:, 0], in_=v_in.ap()[0])
            qkr2 = load_qk(2)
            nc.gpsimd.dma_start(out=vsb[:, 1], in_=v_in.ap()[1])
            qkr3 = load_qk(3)
            nc.gpsimd.dma_start(out=vsb[:, 2], in_=v_in.ap()[2])
            nc.gpsimd.dma_start(out=vsb[:, 3], in_=v_in.ap()[3])

            cp0 = cpr.tile([128, 8, 2 * 128], f16, tag="cp", name="cp0")
            cp1 = cpr.tile([128, 8, 2 * 128], f16, tag="cp", name="cp1")
            phase_a(0, 0, qkr00)
            sc(0, 0)
            phase_a(0, 1, qkr01)
            ct(0, 0, cp0)
            sc(1, 0)
            phase_a(1, 0, qkr1, hofs=0)
            sc(0, 1)
            phase_a(1, 1, qkr1, hofs=1)
            ct(1, 0, cp1)
            phase_a(2, 0, qkr2, hofs=0)
            sc(1, 1)
            ct(0, 1, cp0)
            phase_a(2, 1, qkr2, hofs=1)
            nc.gpsimd.dma_start(out=wt_sb, in_=wt_in.ap())
            sc(0, 2)
            ct(1, 1, cp1)
            phase_a(3, 0, qkr3, hofs=0)
            sc(1, 2)
            phase_a(3, 1, qkr3, hofs=1)
            ct(0, 2, cp0)
            sc(0, 3)
            ct(1, 2, cp1)
            ct(0, 3, cp0, last_head=True, halves=(0,))
            ct(0, 3, cp0, last_head=True, halves=(1,))
            sc(1, 3)
            proj(0, 0)
            proj(0, 1)
            ct(1, 3, cp1, last_head=True)
            proj(1, 0, mixed=True)
            proj(1, 1, mixed=True)
    _legalize_waits(nc)
    return nc


# even rope lanes first, then odd - a consistent permutation of q and k
# features leaves q.k dot products unchanged, so every rope op is a
# contiguous full-width op
_ROPE_PERM = np.concatenate([np.arange(0, D, 2), np.arange(1, D, 2)])


def _shard_inputs(q, k, v, cos, sin, proj_w):
    """Per-core input maps (host-side layout prep only - no module math)."""
    qh = q.reshape(B, S, H, D)
    kh = k.reshape(B, S, H, D)
    vh = v.reshape(B, S, H, D)
    # doubled rope tables [cos|cos], [sin|sin]: [2, S, D]
    cs = np.stack([np.concatenate([cos, cos], 1),
                   np.concatenate([sin, sin], 1)])
    cs_t = np.ascontiguousarray(
        cs.reshape(2, NT, 128, D).transpose(2, 0, 1, 3), np.float32)
    maps = []
    for core in range(N_CORES):
        b, hg = divmod(core, 4)
        hs = slice(HL * hg, HL * (hg + 1))

        def tiles(x, permute):
            xs = x[b][:, hs, :].transpose(1, 0, 2)     # [HL, S, D]
            if permute:
                xs = xs[..., _ROPE_PERM]
            # [NJ, NT, 128, D] -> [NJ, 128, NT, D] partition-major
            return np.ascontiguousarray(
                xs.reshape(NJ, NT, 128, D).transpose(0, 2, 1, 3), np.float32)

        qk_c = np.concatenate([tiles(qh, True), tiles(kh, True)], axis=3)
        v_c = np.concatenate(
            [tiles(vh, False), np.ones((NJ, 128, NT, 1), np.float32)], axis=3)
        fs = slice(256 * hg, 256 * (hg + 1))
        wt_c = np.ascontiguousarray(
            proj_w[:, fs].T.reshape(2, 128, E).transpose(1, 0, 2), np.float32)
        maps.append({
            "qk": np.ascontiguousarray(qk_c),
            "v": np.ascontiguousarray(v_c),
            "cs": cs_t, "wt": wt_c,
        })
    return maps


_NC_CACHE = []


def _get_nc():
    if not _NC_CACHE:
        _NC_CACHE.append(build_nc())
    return _NC_CACHE[0]


def kernel(q, k, v, attn_mask, padding_mask, qn_w, kn_w, proj_w, proj_b,
           cos, sin):
    q = np.asarray(q, np.float32)
    k = np.asarray(k, np.float32)
    v = np.asarray(v, np.float32)
    proj_w = np.asarray(proj_w, np.float32)
    proj_b = np.asarray(proj_b, np.float32)
    cos = np.asarray(cos, np.float32)
    sin = np.asarray(sin, np.float32)
    attn_mask = np.asarray(attn_mask)
    padding_mask = np.asarray(padding_mask)
    qn_w = np.asarray(qn_w, np.float32)
    kn_w = np.asarray(kn_w, np.float32)
    # The kernel bakes in: causal attn_mask, no padding, unit RMSNorm weights.
    assert np.array_equal(
        attn_mask.reshape(S, S), np.tril(np.ones((S, S), attn_mask.dtype)))
    assert padding_mask.all()
    assert np.all(qn_w == 1.0) and np.all(kn_w == 1.0)

    in_maps = _shard_inputs(q, k, v, cos, sin, proj_w)
    nc = _get_nc()
    res = run_bass_kernel_spmd(nc, in_maps, core_ids=list(range(N_CORES)))
    parts = np.stack([r["out"].reshape(S, E) for r in res.results])
    full = np.empty((B, S, E), np.float32)
    for b in range(B):
        full[b] = parts[4 * b:4 * b + 4].astype(np.float32).sum(axis=0)
    return (full + proj_b[None, None, :]).astype(np.float32)


# revision 15
# speedup vs baseline: 1.0318x; 1.0318x over previous
"""Trainium2 Bass kernel for nn_BaseMultiHeadAttention (B=2, S=2048, E=1024, H=16).

Sharding: core = (batch, head-group of 4). Each of the 8 NeuronCores runs the
full attention pipeline for 4 heads of one batch element, computes a partial
output projection over its 256 context features, and writes a [S, E] fp16
partial; the host sums the 4 partials per batch (the all-reduce) and adds the
bias.

Device pipeline per core (all-fp16 compute, fp32 PSUM accumulate):
  Phase A (per head, per 8-tile half): one gpsimd casting DMA loads the
    host-interleaved [q|k] block fp32->fp16; one fused square + one grouped
    reduce give both RMS sums; sqrt (ACT) + reciprocal (DVE). q is
    normalized on DVE; k's scale is folded into the exp scale AP (per-PSUM-
    partition = per k position), so k skips the normalize multiply. RoPE
    uses host-doubled cos/sin tables: 2 full-width muls + add/sub (fp16 2x
    DVE) write q into cols [0:64) and k into cols [64:128) of a padded
    staging slot; one XBAR DMA-transpose yields qkT[128, t, s] (q_d on
    partitions 0-63) and a second, 64-col-shifted one yields kT - no PE
    transposes, no PSUM->SBUF copies.
  Attention (per 1024-q chunk-pair P, per head) split into sc() and ct():
    sc: per k-tile jj, matmuls fill scoresT [128 k, 1024 q] PSUM causal-
    tight (only the q >= jj*128 suffix), exp on ACT (scale AP = D^-0.5 *
    rms_k) into fp16 p tiles, gpsimd affine_select zeroes the diagonal
    triangle. ct: ctx = p.T@[v|1] accumulates in PSUM (ones column = softmax
    row sums), rows scaled by 1/sum on the PSUM read into cpair. P0 needs
    only phase-A half 0, so the exp stream starts early; P0's last heads run
    at the end so the drain tail is the small chunk. Emission order is hand-
    pipelined per engine (streams are in-order).
  Projection (per 128-row tile): XBAR DMA-transpose cpair -> ctxT [128,2,128]
    prefetched right after the last head's normalize, 2x2 matmuls against
    fp16 wt, PSUM->SBUF fp16 copies (DVE/Pool alternating), fp16 DMA out.
"""
import numpy as np

import bass_rust
import concourse.bass as bass
import concourse.mybir as mybir
import concourse.tile as tile
from concourse.bass_utils import run_bass_kernel_spmd

B, S, E, H, D = 2, 2048, 1024, 16, 64
HD = D // 2
N_CORES = 8
HL = 4                     # heads per core
NJ = HL                    # jobs per core = heads (single batch per core)
NT = S // 128              # 16 s-tiles
NH = NT // 2               # tiles per phase-A half
EPS = 1.1920928955078125e-07
f32 = mybir.dt.float32
f16 = mybir.dt.float16
ALU = mybir.AluOpType
ACTF = mybir.ActivationFunctionType
AXX = mybir.AxisListType.X

_TC = tile.TileContext


def _legalize_waits(nc):
    """Split multi-wait sync_infos for this walrus build.

    This neuronxcc's codegen allows 1 sync wait per instruction (2 on
    EventSemaphore), while the Tile scheduler attaches all outstanding
    waits to one instruction.  Hoist the excess onto same-engine NoOps
    inserted immediately before the offending instruction - the engine
    executes its stream in order, so blocking semantics are identical.
    """
    uid = 0
    for f in nc.m.functions:
        for blk in f.blocks:
            insts = list(blk.instructions)
            out, changed = [], False
            for inst in insts:
                si = inst.sync_info
                cap = 2 if isinstance(inst, mybir.InstEventSemaphore) else 1
                if si is not None and len(si.on_wait) > cap:
                    changed = True
                    waits = list(si.on_wait)
                    for w in waits[:-cap]:
                        carrier = mybir.InstNoOp(
                            name=f"legwait-{uid}", engine=inst.engine,
                            ins=[], outs=[])
                        uid += 1
                        carrier.sync_info = bass_rust.SyncInfo(
                            on_wait=[w], on_update=[])
                        nc.register_instruction(carrier, overwrite=True)
                        out.append(carrier)
                    si.on_wait = waits[-cap:]
                    inst.sync_info = si
                out.append(inst)
            if changed:
                blk.instructions = out


def _flat_cols(src, start, count):
    """AP over a [128, ...] slice viewed as flat columns [start, start+count)."""
    return bass.AP(tensor=src.tensor, offset=src.offset + start,
                   ap=[list(src.ap[0]), [1, count]])


def build_nc():
    nc = bass.Bass("TRN2", target_bir_lowering=False, debug=False)
    qk_in = nc.dram_tensor("qk", [NJ, 128, NT, 128], f32, kind="ExternalInput")
    v_in = nc.dram_tensor("v", [NJ, 128, NT, D + 1], f32, kind="ExternalInput")
    cs_in = nc.dram_tensor("cs", [128, 2, NT, D], f32, kind="ExternalInput")
    wt_in = nc.dram_tensor("wt", [128, 2, E], f32, kind="ExternalInput")
    out = nc.dram_tensor("out", [NT, 128, E], f16, kind="ExternalOutput")

    with _TC(nc) as tc, nc.allow_low_precision(reason="fp16 attention"):
        with tc.tile_pool(name="const", bufs=1) as cp, \
             tc.tile_pool(name="pl", bufs=5) as pl, \
             tc.tile_pool(name="pa", bufs=3) as pa, \
             tc.tile_pool(name="pp", bufs=34) as pp, \
             tc.tile_pool(name="pb", bufs=4) as pb, \
             tc.tile_pool(name="pt", bufs=10) as ptp, \
             tc.tile_pool(name="po", bufs=4) as po, \
             tc.tile_pool(name="cpr", bufs=2) as cpr, \
             tc.tile_pool(name="ps_s", bufs=2, space="PSUM") as ps_s, \
             tc.tile_pool(name="ps_c", bufs=2, space="PSUM") as ps_c, \
             tc.tile_pool(name="ps_o", bufs=2, space="PSUM") as ps_o:
            eps_t = cp.tile([128, 1], f32)
            nc.vector.memset(eps_t, EPS)
            eps64_t = cp.tile([128, 1], f32)
            nc.vector.memset(eps64_t, D * EPS)
            cs_sb = cp.tile([128, 2, NT, D], f16)   # doubled [cos|cos],[sin|sin]
            wt_sb = cp.tile([128, 2, E], f16)
            vsb = cp.tile([128, NJ, NT, D + 1], f16)
            qkin = cp.tile([128, 2, NT + 1, 128], f16)   # 2-slot ring by job
            qkT = cp.tile([128, NJ, NT, 128], f16)
            kT = cp.tile([128, NJ, NT, 128], f16)
            rs2 = cp.tile([128, NJ, NT, 2], f32)    # interleaved rs_q, rs_k

            for sl_ in range(2):
                # junk pad block read by the shifted k transpose
                nc.gpsimd.memset(qkin[:, sl_, NT, :], 0.0)

            # ---------------- Phase A: norm + rope + transposes ------------
            def load_qk(j, h=None):
                ts = slice(0, NT) if h is None else slice(h * NH, (h + 1) * NH)
                n = ts.stop - ts.start
                qkr = pl.tile([128, n, 2, D], f16, tag="qkr", name="qkr")
                nc.gpsimd.dma_start(out=qkr, in_=qk_in.ap()[j][:, ts])
                return qkr

            def phase_a(j, h, qkr, hofs=0):
                ts = slice(h * NH, (h + 1) * NH)
                to = slice(hofs * NH, hofs * NH + NH)
                qr = qkr[:, to, 0, :]
                kr = qkr[:, to, 1, :]
                sq = pa.tile([128, NH, 2, D], f16, tag="sq", name="sq")
                nc.vector.tensor_mul(sq, qkr[:, to], qkr[:, to])
                rs_sl = rs2[:, j, ts, :]
                nc.vector.reduce_sum(rs_sl, sq, axis=AXX)
                # rs_q = 1/sqrt(ss/D + eps); rs_k = D^-0.5/sqrt(ss/D + eps)
                # = 1/sqrt(ss + D*eps), folded into the exp scale AP
                nc.scalar.activation(
                    out=rs2[:, j, ts, 0], in_=rs2[:, j, ts, 0],
                    func=ACTF.Sqrt, bias=eps_t, scale=1.0 / D)
                nc.scalar.activation(
                    out=rs2[:, j, ts, 1], in_=rs2[:, j, ts, 1],
                    func=ACTF.Sqrt, bias=eps64_t, scale=1.0)
                nc.vector.reciprocal(out=rs_sl, in_=rs_sl)
                xq = pa.tile([128, NH, D], f16, tag="xq", name="xq")
                rsb = bass.AP(
                    tensor=rs2.tensor,
                    offset=rs2.offset + (j * NT + ts.start) * 2,
                    ap=[list(rs2.ap[0]), [2, NH], [0, D]])
                nc.vector.tensor_mul(xq, qr, rsb)
                csl, ssl = cs_sb[:, 0, ts, :], cs_sb[:, 1, ts, :]
                sl_ = j % 2
                for src, off in ((xq, 0), (kr, 64)):
                    # src cols = [x1|x2]; doubled tables: tc=[x1c|x2c],
                    # tsn=[x1s|x2s]; o1 = x1c - x2s; o2 = x1s + x2c
                    tc = pa.tile([128, NH, D], f16, tag="tc", name="tc")
                    tsn = pa.tile([128, NH, D], f16, tag="tsn", name="tsn")
                    nc.vector.tensor_mul(tc, src, csl)
                    nc.vector.tensor_mul(tsn, src, ssl)
                    nc.vector.tensor_sub(
                        qkin[:, sl_, ts, off:off + HD],
                        tc[:, :, 0:HD], tsn[:, :, HD:D])
                    nc.vector.tensor_add(
                        qkin[:, sl_, ts, off + HD:off + D],
                        tsn[:, :, 0:HD], tc[:, :, HD:D])
                src2d = qkin[:, sl_, 0, 0:1]
                nc.sync.dma_start_transpose(
                    out=qkT[:, j, ts, :],
                    in_=_flat_cols(src2d, ts.start * 128, NH * 128))
                nc.sync.dma_start_transpose(
                    out=kT[:, j, ts, :],
                    in_=_flat_cols(src2d, ts.start * 128 + 64, NH * 128))

            # ---------------- Attention scores+exp for chunk-pair P --------
            pts_map = {}

            def sc(P, j):
                njj = 8 * P + 8
                pts = []
                for jj in range(njj):
                    lo = max(jj * 128 - 1024 * P, 0)
                    sps = ps_s.tile([128, 1024], f32, tag="s", name="sps")
                    pieces = [(lo, 512), (512, 1024)] if lo < 512 \
                        else [(lo, 1024)]
                    for a, b in pieces:
                        nc.tensor.matmul(
                            sps[:, a:b],
                            lhsT=kT[0:64, j, jj, :],
                            rhs=qkT[0:64, j,
                                    8 * P + a // 128:8 * P + b // 128, :],
                            start=True, stop=True)
                    pt = pp.tile([128, 1024], f16, tag="p", name="pt")
                    nc.scalar.activation(
                        out=pt[:, lo:1024], in_=sps[:, lo:1024],
                        func=ACTF.Exp, scale=rs2[:, j, jj, 1:2])
                    if jj >= 8 * P:
                        psl = pt[:, lo:lo + 128]
                        nc.gpsimd.affine_select(
                            out=psl, in_=psl, compare_op=ALU.is_ge,
                            fill=0.0, base=0, pattern=[[1, 128]],
                            channel_multiplier=-1)
                    pts.append(pt)
                pts_map[(P, j)] = pts

            # ---------------- ctx + normalize (+ ctxT prefetch) ------------
            ctxT_tiles = {}

            def ct(P, j, cp_t, last_head=False, halves=(0, 1)):
                pts = pts_map[(P, j)]
                for half in halves:
                    ctxp = ps_c.tile([128, 4, D + 1], f32, tag="c",
                                     name="ctxp")
                    for qb in range(4):
                        i = 8 * P + 4 * half + qb
                        ic = (4 * half + qb) * 128
                        for jj in range(i + 1):
                            nc.tensor.matmul(
                                ctxp[:, qb, :],
                                lhsT=pts[jj][:, ic:ic + 128],
                                rhs=vsb[:, j, jj, :],
                                start=(jj == 0), stop=(jj == i))
                    if not last_head:
                        rsum = pb.tile([128, 4], f32, tag="rsum",
                                       name="rsum")
                        nc.vector.reciprocal(out=rsum, in_=ctxp[:, :, D])
                        rsum_b = bass.AP(
                            tensor=rsum.tensor, offset=rsum.offset,
                            ap=[list(rsum.ap[0]), [1, 4], [0, D]])
                        nc.vector.tensor_mul(
                            cp_t[:, 4 * half:4 * half + 4,
                                 64 * j:64 * j + 64],
                            ctxp[:, :, 0:D], rsum_b)
                    else:
                        # last head: per-2-tile normalize + transpose so the
                        # proj chain starts before the whole half is done
                        ctxT4 = ptp.tile([128, 8, 128], f16, tag="ctxT",
                                         name="ctxT4")
                        for pr in range(2):
                            t2 = slice(4 * half + 2 * pr,
                                       4 * half + 2 * pr + 2)
                            rsum = pb.tile([128, 2], f32, tag="rsum",
                                           name="rsum")
                            nc.vector.reciprocal(
                                out=rsum, in_=ctxp[:, 2 * pr:2 * pr + 2, D])
                            rsum_b = bass.AP(
                                tensor=rsum.tensor, offset=rsum.offset,
                                ap=[list(rsum.ap[0]), [1, 2], [0, D]])
                            nc.vector.tensor_mul(
                                cp_t[:, t2, 64 * j:64 * j + 64],
                                ctxp[:, 2 * pr:2 * pr + 2, 0:D], rsum_b)
                            csrc = cp_t[:, t2.start, 0:1]
                            nc.sync.dma_start_transpose(
                                out=ctxT4[:, 4 * pr:4 * pr + 4, :],
                                in_=_flat_cols(csrc, 0, 512))
                        ctxT_tiles[(P, half)] = ctxT4
                if last_head and halves[-1] == 1:
                    del pts_map[(P, j)]

            # ---------------- Projection for s-tiles of chunk-pair P -------
            def proj(P, half, mixed=False):
                ctxT4 = ctxT_tiles.pop((P, half))
                for pair in range(2):
                    osb = po.tile([128, 2, E], f16, tag="osb", name="osb")
                    for tp in range(2):
                        ip = 2 * pair + tp
                        for eh in range(2):
                            pro = ps_o.tile([128, 512], f32, tag="o",
                                            name="pro")
                            nc.tensor.matmul(
                                pro, lhsT=ctxT4[:, 2 * ip, :],
                                rhs=wt_sb[:, 0, eh * 512:(eh + 1) * 512],
                                start=True, stop=False)
                            nc.tensor.matmul(
                                pro, lhsT=ctxT4[:, 2 * ip + 1, :],
                                rhs=wt_sb[:, 1, eh * 512:(eh + 1) * 512],
                                start=False, stop=True)
                            dst = osb[:, tp, eh * 512:(eh + 1) * 512]
                            if mixed and (tp + eh) % 2 == 0:
                                nc.scalar.copy(dst, pro)
                            else:
                                nc.vector.tensor_copy(dst, pro)
                    i0 = 8 * P + 4 * half + 2 * pair
                    nc.sync.dma_start(
                        out=out.ap()[i0:i0 + 2].rearrange("t p e -> p t e"),
                        in_=osb)

            # ---------------- Emission schedule ----------------------------
            # gpsimd casts first (Pool is in-order; selects would starve
            # them); compute emission is hand-pipelined so that no engine's
            # in-order stream blocks work another engine needs soon.
            qkr00 = load_qk(0, 0)
            nc.gpsimd.dma_start(out=cs_sb, in_=cs_in.ap())
            qkr01 = load_qk(0, 1)
            qkr1 = load_qk(1)
            nc.gpsimd.dma_start(out=vsb[:, 0], in_=v_in.ap()[0])
            qkr2 = load_qk(2)
            nc.gpsimd.dma_start(out=vsb[:, 1], in_=v_in.ap()[1])
            qkr3 = load_qk(3)
            nc.gpsimd.dma_start(out=vsb[:, 2], in_=v_in.ap()[2])
            nc.gpsimd.dma_start(out=vsb[:, 3], in_=v_in.ap()[3])

            cp0 = cpr.tile([128, 8, 2 * 128], f16, tag="cp", name="cp0")
            cp1 = cpr.tile([128, 8, 2 * 128], f16, tag="cp", name="cp1")
            phase_a(0, 0, qkr00)
            sc(0, 0)
            phase_a(0, 1, qkr01)
            ct(0, 0, cp0)
            sc(1, 0)
            phase_a(1, 0, qkr1, hofs=0)
            sc(0, 1)
            phase_a(1, 1, qkr1, hofs=1)
            ct(1, 0, cp1)
            phase_a(2, 0, qkr2, hofs=0)
            sc(1, 1)
            ct(0, 1, cp0)
            phase_a(2, 1, qkr2, hofs=1)
            nc.gpsimd.dma_start(out=wt_sb, in_=wt_in.ap())
            sc(0, 2)
            ct(1, 1, cp1)
            phase_a(3, 0, qkr3, hofs=0)
            sc(1, 2)
            phase_a(3, 1, qkr3, hofs=1)
            ct(0, 2, cp0)
            sc(0, 3)
            ct(1, 2, cp1)
            ct(0, 3, cp0, last_head=True, halves=(0,))
            ct(0, 3, cp0, last_head=True, halves=(1,))
            sc(1, 3)
            ct(1, 3, cp1, last_head=True)
            proj(0, 0)
            proj(0, 1)
            proj(1, 0, mixed=True)
            proj(1, 1, mixed=True)
    _legalize_waits(nc)
    return nc


# even rope lanes first, then odd - a consistent permutation of q and k
# features leaves q.k dot products unchanged, so every rope op is a
# contiguous full-width op
_ROPE_PERM = np.concatenate([np.arange(0, D, 2), np.arange(1, D, 2)])


def _shard_inputs(q, k, v, cos, sin, proj_w):
    """Per-core input maps (host-side layout prep only - no module math)."""
    qh = q.reshape(B, S, H, D)
    kh = k.reshape(B, S, H, D)
    vh = v.reshape(B, S, H, D)
    # doubled rope tables [cos|cos], [sin|sin]: [2, S, D]
    cs = np.stack([np.concatenate([cos, cos], 1),
                   np.concatenate([sin, sin], 1)])
    cs_t = np.ascontiguousarray(
        cs.reshape(2, NT, 128, D).transpose(2, 0, 1, 3), np.float32)
    maps = []
    for core in range(N_CORES):
        b, hg = divmod(core, 4)
        hs = slice(HL * hg, HL * (hg + 1))

        def tiles(x, permute):
            xs = x[b][:, hs, :].transpose(1, 0, 2)     # [HL, S, D]
            if permute:
                xs = xs[..., _ROPE_PERM]
            # [NJ, NT, 128, D] -> [NJ, 128, NT, D] partition-major
            return np.ascontiguousarray(
                xs.reshape(NJ, NT, 128, D).transpose(0, 2, 1, 3), np.float32)

        qk_c = np.concatenate([tiles(qh, True), tiles(kh, True)], axis=3)
        v_c = np.concatenate(
            [tiles(vh, False), np.ones((NJ, 128, NT, 1), np.float32)], axis=3)
        fs = slice(256 * hg, 256 * (hg + 1))
        wt_c = np.ascontiguousarray(
            proj_w[:, fs].T.reshape(2, 128, E).transpose(1, 0, 2), np.float32)
        maps.append({
            "qk": np.ascontiguousarray(qk_c),
            "v": np.ascontiguousarray(v_c),
            "cs": cs_t, "wt": wt_c,
        })
    return maps


_NC_CACHE = []


def _get_nc():
    if not _NC_CACHE:
        _NC_CACHE.append(build_nc())
    return _NC_CACHE[0]


def kernel(q, k, v, attn_mask, padding_mask, qn_w, kn_w, proj_w, proj_b,
           cos, sin):
    q = np.asarray(q, np.float32)
    k = np.asarray(k, np.float32)
    v = np.asarray(v, np.float32)
    proj_w = np.asarray(proj_w, np.float32)
    proj_b = np.asarray(proj_b, np.float32)
    cos = np.asarray(cos, np.float32)
    sin = np.asarray(sin, np.float32)
    attn_mask = np.asarray(attn_mask)
    padding_mask = np.asarray(padding_mask)
    qn_w = np.asarray(qn_w, np.float32)
    kn_w = np.asarray(kn_w, np.float32)
    # The kernel bakes in: causal attn_mask, no padding, unit RMSNorm weights.
    assert np.array_equal(
        attn_mask.reshape(S, S), np.tril(np.ones((S, S), attn_mask.dtype)))
    assert padding_mask.all()
    assert np.all(qn_w == 1.0) and np.all(kn_w == 1.0)

    in_maps = _shard_inputs(q, k, v, cos, sin, proj_w)
    nc = _get_nc()
    res = run_bass_kernel_spmd(nc, in_maps, core_ids=list(range(N_CORES)))
    parts = np.stack([r["out"].reshape(S, E) for r in res.results])
    full = np.empty((B, S, E), np.float32)
    for b in range(B):
        full[b] = parts[4 * b:4 * b + 4].astype(np.float32).sum(axis=0)
    return (full + proj_b[None, None, :]).astype(np.float32)


# revision 16
# speedup vs baseline: 1.0385x; 1.0065x over previous
"""Trainium2 Bass kernel for nn_BaseMultiHeadAttention (B=2, S=2048, E=1024, H=16).

Sharding: core = (batch, head-group of 4). Each of the 8 NeuronCores runs the
full attention pipeline for 4 heads of one batch element, computes a partial
output projection over its 256 context features, and writes a [S, E] fp16
partial; the host sums the 4 partials per batch (the all-reduce) and adds the
bias.

Device pipeline per core (all-fp16 compute, fp32 PSUM accumulate):
  Phase A (per head, per 8-tile half): one gpsimd casting DMA loads the
    host-interleaved [q|k] block fp32->fp16; one fused square + one grouped
    reduce give both RMS sums; sqrt (ACT) + reciprocal (DVE). q is
    normalized on DVE; k's scale is folded into the exp scale AP (per-PSUM-
    partition = per k position), so k skips the normalize multiply. RoPE
    uses host-doubled cos/sin tables: 2 full-width muls + add/sub (fp16 2x
    DVE) write q into cols [0:64) and k into cols [64:128) of a padded
    staging slot; one XBAR DMA-transpose yields qkT[128, t, s] (q_d on
    partitions 0-63) and a second, 64-col-shifted one yields kT - no PE
    transposes, no PSUM->SBUF copies.
  Attention (per 1024-q chunk-pair P, per head) split into sc() and ct():
    sc: per k-tile jj, matmuls fill scoresT [128 k, 1024 q] PSUM causal-
    tight (only the q >= jj*128 suffix), exp on ACT (scale AP = D^-0.5 *
    rms_k) into fp16 p tiles, gpsimd affine_select zeroes the diagonal
    triangle. ct: ctx = p.T@[v|1] accumulates in PSUM (ones column = softmax
    row sums), rows scaled by 1/sum on the PSUM read into cpair. P0 needs
    only phase-A half 0, so the exp stream starts early; P0's last heads run
    at the end so the drain tail is the small chunk. Emission order is hand-
    pipelined per engine (streams are in-order).
  Projection (per 128-row tile): XBAR DMA-transpose cpair -> ctxT [128,2,128]
    prefetched right after the last head's normalize, 2x2 matmuls against
    fp16 wt, PSUM->SBUF fp16 copies (DVE/Pool alternating), fp16 DMA out.
"""
import numpy as np

import bass_rust
import concourse.bass as bass
import concourse.mybir as mybir
import concourse.tile as tile
from concourse.bass_utils import run_bass_kernel_spmd

B, S, E, H, D = 2, 2048, 1024, 16, 64
HD = D // 2
N_CORES = 8
HL = 4                     # heads per core
NJ = HL                    # jobs per core = heads (single batch per core)
NT = S // 128              # 16 s-tiles
NH = NT // 2               # tiles per phase-A half
EPS = 1.1920928955078125e-07
f32 = mybir.dt.float32
f16 = mybir.dt.float16
ALU = mybir.AluOpType
ACTF = mybir.ActivationFunctionType
AXX = mybir.AxisListType.X

_TC = tile.TileContext


def _legalize_waits(nc):
    """Split multi-wait sync_infos for this walrus build.

    This neuronxcc's codegen allows 1 sync wait per instruction (2 on
    EventSemaphore), while the Tile scheduler attaches all outstanding
    waits to one instruction.  Hoist the excess onto same-engine NoOps
    inserted immediately before the offending instruction - the engine
    executes its stream in order, so blocking semantics are identical.
    """
    uid = 0
    for f in nc.m.functions:
        for blk in f.blocks:
            insts = list(blk.instructions)
            out, changed = [], False
            for inst in insts:
                si = inst.sync_info
                cap = 2 if isinstance(inst, mybir.InstEventSemaphore) else 1
                if si is not None and len(si.on_wait) > cap:
                    changed = True
                    waits = list(si.on_wait)
                    for w in waits[:-cap]:
                        carrier = mybir.InstNoOp(
                            name=f"legwait-{uid}", engine=inst.engine,
                            ins=[], outs=[])
                        uid += 1
                        carrier.sync_info = bass_rust.SyncInfo(
                            on_wait=[w], on_update=[])
                        nc.register_instruction(carrier, overwrite=True)
                        out.append(carrier)
                    si.on_wait = waits[-cap:]
                    inst.sync_info = si
                out.append(inst)
            if changed:
                blk.instructions = out


def _flat_cols(src, start, count):
    """AP over a [128, ...] slice viewed as flat columns [start, start+count)."""
    return bass.AP(tensor=src.tensor, offset=src.offset + start,
                   ap=[list(src.ap[0]), [1, count]])


def build_nc():
    nc = bass.Bass("TRN2", target_bir_lowering=False, debug=False)
    qk_in = nc.dram_tensor("qk", [NJ, 128, NT, 128], f32, kind="ExternalInput")
    v_in = nc.dram_tensor("v", [NJ, 128, NT, D + 1], f32, kind="ExternalInput")
    cs_in = nc.dram_tensor("cs", [128, 2, NT, D], f32, kind="ExternalInput")
    wt_in = nc.dram_tensor("wt", [128, 2, E], f32, kind="ExternalInput")
    out = nc.dram_tensor("out", [NT, 128, E], f16, kind="ExternalOutput")

    with _TC(nc) as tc, nc.allow_low_precision(reason="fp16 attention"):
        with tc.tile_pool(name="const", bufs=1) as cp, \
             tc.tile_pool(name="pl", bufs=5) as pl, \
             tc.tile_pool(name="pa", bufs=3) as pa, \
             tc.tile_pool(name="pp", bufs=34) as pp, \
             tc.tile_pool(name="pb", bufs=4) as pb, \
             tc.tile_pool(name="pt", bufs=10) as ptp, \
             tc.tile_pool(name="po", bufs=4) as po, \
             tc.tile_pool(name="cpr", bufs=2) as cpr, \
             tc.tile_pool(name="ps_s", bufs=2, space="PSUM") as ps_s, \
             tc.tile_pool(name="ps_c", bufs=2, space="PSUM") as ps_c, \
             tc.tile_pool(name="ps_o", bufs=2, space="PSUM") as ps_o:
            eps_t = cp.tile([128, 1], f32)
            nc.vector.memset(eps_t, EPS)
            eps64_t = cp.tile([128, 1], f32)
            nc.vector.memset(eps64_t, D * EPS)
            cs_sb = cp.tile([128, 2, NT, D], f16)   # doubled [cos|cos],[sin|sin]
            wt_sb = cp.tile([128, 2, E], f16)
            vsb = cp.tile([128, NJ, NT, D + 1], f16)
            qkin = cp.tile([128, 2, NT + 1, 128], f16)   # 2-slot ring by job
            qkT = cp.tile([128, NJ, NT, 128], f16)
            kT = cp.tile([128, NJ, NT, 128], f16)
            rs2 = cp.tile([128, NJ, NT, 2], f32)    # interleaved rs_q, rs_k

            for sl_ in range(2):
                # junk pad block read by the shifted k transpose
                nc.gpsimd.memset(qkin[:, sl_, NT, :], 0.0)

            # ---------------- Phase A: norm + rope + transposes ------------
            def load_qk(j, h=None):
                ts = slice(0, NT) if h is None else slice(h * NH, (h + 1) * NH)
                n = ts.stop - ts.start
                qkr = pl.tile([128, n, 2, D], f16, tag="qkr", name="qkr")
                nc.gpsimd.dma_start(out=qkr, in_=qk_in.ap()[j][:, ts])
                return qkr

            def phase_a(j, h, qkr, hofs=0):
                ts = slice(h * NH, (h + 1) * NH)
                to = slice(hofs * NH, hofs * NH + NH)
                qr = qkr[:, to, 0, :]
                kr = qkr[:, to, 1, :]
                sq = pa.tile([128, NH, 2, D], f16, tag="sq", name="sq")
                nc.vector.tensor_mul(sq, qkr[:, to], qkr[:, to])
                rs_sl = rs2[:, j, ts, :]
                nc.vector.reduce_sum(rs_sl, sq, axis=AXX)
                # rs_q = 1/sqrt(ss/D + eps); rs_k = D^-0.5/sqrt(ss/D + eps)
                # = 1/sqrt(ss + D*eps), folded into the exp scale AP
                nc.scalar.activation(
                    out=rs2[:, j, ts, 0], in_=rs2[:, j, ts, 0],
                    func=ACTF.Sqrt, bias=eps_t, scale=1.0 / D)
                nc.scalar.activation(
                    out=rs2[:, j, ts, 1], in_=rs2[:, j, ts, 1],
                    func=ACTF.Sqrt, bias=eps64_t, scale=1.0)
                nc.vector.reciprocal(out=rs_sl, in_=rs_sl)
                xq = pa.tile([128, NH, D], f16, tag="xq", name="xq")
                rsb = bass.AP(
                    tensor=rs2.tensor,
                    offset=rs2.offset + (j * NT + ts.start) * 2,
                    ap=[list(rs2.ap[0]), [2, NH], [0, D]])
                nc.vector.tensor_mul(xq, qr, rsb)
                csl, ssl = cs_sb[:, 0, ts, :], cs_sb[:, 1, ts, :]
                sl_ = j % 2
                for src, off in ((xq, 0), (kr, 64)):
                    # src cols = [x1|x2]; doubled tables: tc=[x1c|x2c],
                    # tsn=[x1s|x2s]; o1 = x1c - x2s; o2 = x1s + x2c
                    tc = pa.tile([128, NH, D], f16, tag="tc", name="tc")
                    tsn = pa.tile([128, NH, D], f16, tag="tsn", name="tsn")
                    nc.vector.tensor_mul(tc, src, csl)
                    nc.vector.tensor_mul(tsn, src, ssl)
                    nc.vector.tensor_sub(
                        qkin[:, sl_, ts, off:off + HD],
                        tc[:, :, 0:HD], tsn[:, :, HD:D])
                    nc.vector.tensor_add(
                        qkin[:, sl_, ts, off + HD:off + D],
                        tsn[:, :, 0:HD], tc[:, :, HD:D])
                src2d = qkin[:, sl_, 0, 0:1]
                nc.sync.dma_start_transpose(
                    out=qkT[:, j, ts, :],
                    in_=_flat_cols(src2d, ts.start * 128, NH * 128))
                nc.sync.dma_start_transpose(
                    out=kT[:, j, ts, :],
                    in_=_flat_cols(src2d, ts.start * 128 + 64, NH * 128))

            # ---------------- Attention scores+exp for chunk-pair P --------
            pts_map = {}

            def sc(P, j):
                njj = 8 * P + 8
                pts = []
                for jj in range(njj):
                    lo = max(jj * 128 - 1024 * P, 0)
                    sps = ps_s.tile([128, 1024], f32, tag="s", name="sps")
                    pieces = [(lo, 512), (512, 1024)] if lo < 512 \
                        else [(lo, 1024)]
                    for a, b in pieces:
                        nc.tensor.matmul(
                            sps[:, a:b],
                            lhsT=kT[0:64, j, jj, :],
                            rhs=qkT[0:64, j,
                                    8 * P + a // 128:8 * P + b // 128, :],
                            start=True, stop=True)
                    pt = pp.tile([128, 1024], f16, tag="p", name="pt")
                    nc.scalar.activation(
                        out=pt[:, lo:1024], in_=sps[:, lo:1024],
                        func=ACTF.Exp, scale=rs2[:, j, jj, 1:2])
                    if jj >= 8 * P:
                        psl = pt[:, lo:lo + 128]
                        nc.gpsimd.affine_select(
                            out=psl, in_=psl, compare_op=ALU.is_ge,
                            fill=0.0, base=0, pattern=[[1, 128]],
                            channel_multiplier=-1)
                    pts.append(pt)
                pts_map[(P, j)] = pts

            # ---------------- ctx + normalize (+ ctxT prefetch) ------------
            ctxT_tiles = {}

            def ct(P, j, cp_t, last_head=False, halves=(0, 1)):
                pts = pts_map[(P, j)]
                for half in halves:
                    ctxp = ps_c.tile([128, 4, D + 1], f32, tag="c",
                                     name="ctxp")
                    for qb in range(4):
                        i = 8 * P + 4 * half + qb
                        ic = (4 * half + qb) * 128
                        for jj in range(i + 1):
                            nc.tensor.matmul(
                                ctxp[:, qb, :],
                                lhsT=pts[jj][:, ic:ic + 128],
                                rhs=vsb[:, j, jj, :],
                                start=(jj == 0), stop=(jj == i))
                    if not last_head:
                        rsum = pb.tile([128, 4], f32, tag="rsum",
                                       name="rsum")
                        nc.vector.reciprocal(out=rsum, in_=ctxp[:, :, D])
                        rsum_b = bass.AP(
                            tensor=rsum.tensor, offset=rsum.offset,
                            ap=[list(rsum.ap[0]), [1, 4], [0, D]])
                        nc.vector.tensor_mul(
                            cp_t[:, 4 * half:4 * half + 4,
                                 64 * j:64 * j + 64],
                            ctxp[:, :, 0:D], rsum_b)
                    else:
                        # last head: per-2-tile normalize + transpose so the
                        # proj chain starts before the whole half is done
                        ctxT4 = ptp.tile([128, 8, 128], f16, tag="ctxT",
                                         name="ctxT4")
                        for pr in range(2):
                            t2 = slice(4 * half + 2 * pr,
                                       4 * half + 2 * pr + 2)
                            rsum = pb.tile([128, 2], f32, tag="rsum",
                                           name="rsum")
                            nc.vector.reciprocal(
                                out=rsum, in_=ctxp[:, 2 * pr:2 * pr + 2, D])
                            rsum_b = bass.AP(
                                tensor=rsum.tensor, offset=rsum.offset,
                                ap=[list(rsum.ap[0]), [1, 2], [0, D]])
                            nc.vector.tensor_mul(
                                cp_t[:, t2, 64 * j:64 * j + 64],
                                ctxp[:, 2 * pr:2 * pr + 2, 0:D], rsum_b)
                            csrc = cp_t[:, t2.start, 0:1]
                            nc.sync.dma_start_transpose(
                                out=ctxT4[:, 4 * pr:4 * pr + 4, :],
                                in_=_flat_cols(csrc, 0, 512))
                        ctxT_tiles[(P, half)] = ctxT4
                if last_head and halves[-1] == 1:
                    del pts_map[(P, j)]

            # ---------------- Projection for s-tiles of chunk-pair P -------
            def proj(P, half, mixed=False):
                ctxT4 = ctxT_tiles.pop((P, half))
                for pair in range(2):
                    osb = po.tile([128, 2, E], f16, tag="osb", name="osb")
                    for tp in range(2):
                        ip = 2 * pair + tp
                        for eh in range(2):
                            pro = ps_o.tile([128, 512], f32, tag="o",
                                            name="pro")
                            nc.tensor.matmul(
                                pro, lhsT=ctxT4[:, 2 * ip, :],
                                rhs=wt_sb[:, 0, eh * 512:(eh + 1) * 512],
                                start=True, stop=False)
                            nc.tensor.matmul(
                                pro, lhsT=ctxT4[:, 2 * ip + 1, :],
                                rhs=wt_sb[:, 1, eh * 512:(eh + 1) * 512],
                                start=False, stop=True)
                            dst = osb[:, tp, eh * 512:(eh + 1) * 512]
                            if mixed and (tp + eh) % 2 == 0:
                                nc.scalar.copy(dst, pro)
                            else:
                                nc.vector.tensor_copy(dst, pro)
                    i0 = 8 * P + 4 * half + 2 * pair
                    nc.sync.dma_start(
                        out=out.ap()[i0:i0 + 2].rearrange("t p e -> p t e"),
                        in_=osb)

            # ---------------- Emission schedule ----------------------------
            # gpsimd casts first (Pool is in-order; selects would starve
            # them); compute emission is hand-pipelined so that no engine's
            # in-order stream blocks work another engine needs soon.
            qkr00 = load_qk(0, 0)
            nc.gpsimd.dma_start(out=cs_sb, in_=cs_in.ap())
            qkr01 = load_qk(0, 1)
            qkr1 = load_qk(1)
            nc.gpsimd.dma_start(out=vsb[:, 0], in_=v_in.ap()[0])
            qkr2 = load_qk(2)
            nc.gpsimd.dma_start(out=vsb[:, 1], in_=v_in.ap()[1])
            qkr3 = load_qk(3)
            nc.gpsimd.dma_start(out=vsb[:, 2], in_=v_in.ap()[2])
            nc.gpsimd.dma_start(out=vsb[:, 3], in_=v_in.ap()[3])

            cp0 = cpr.tile([128, 8, 2 * 128], f16, tag="cp", name="cp0")
            cp1 = cpr.tile([128, 8, 2 * 128], f16, tag="cp", name="cp1")
            phase_a(0, 0, qkr00)
            sc(0, 0)
            phase_a(1, 0, qkr1, hofs=0)
            ct(0, 0, cp0)
            sc(0, 1)
            phase_a(0, 1, qkr01)
            sc(1, 0)
            phase_a(1, 1, qkr1, hofs=1)
            ct(0, 1, cp0)
            phase_a(2, 0, qkr2, hofs=0)
            sc(1, 1)
            ct(1, 0, cp1)
            phase_a(2, 1, qkr2, hofs=1)
            nc.gpsimd.dma_start(out=wt_sb, in_=wt_in.ap())
            sc(0, 2)
            ct(1, 1, cp1)
            phase_a(3, 0, qkr3, hofs=0)
            sc(1, 2)
            phase_a(3, 1, qkr3, hofs=1)
            ct(0, 2, cp0)
            sc(0, 3)
            ct(1, 2, cp1)
            ct(0, 3, cp0, last_head=True, halves=(0,))
            ct(0, 3, cp0, last_head=True, halves=(1,))
            sc(1, 3)
            ct(1, 3, cp1, last_head=True)
            proj(0, 0)
            proj(0, 1)
            proj(1, 0, mixed=True)
            proj(1, 1, mixed=True)
    _legalize_waits(nc)
    return nc


# even rope lanes first, then odd - a consistent permutation of q and k
# features leaves q.k dot products unchanged, so every rope op is a
# contiguous full-width op
_ROPE_PERM = np.concatenate([np.arange(0, D, 2), np.arange(1, D, 2)])


def _shard_inputs(q, k, v, cos, sin, proj_w):
    """Per-core input maps (host-side layout prep only - no module math)."""
    qh = q.reshape(B, S, H, D)
    kh = k.reshape(B, S, H, D)
    vh = v.reshape(B, S, H, D)
    # doubled rope tables [cos|cos], [sin|sin]: [2, S, D]
    cs = np.stack([np.concatenate([cos, cos], 1),
                   np.concatenate([sin, sin], 1)])
    cs_t = np.ascontiguousarray(
        cs.reshape(2, NT, 128, D).transpose(2, 0, 1, 3), np.float32)
    maps = []
    for core in range(N_CORES):
        b, hg = divmod(core, 4)
        hs = slice(HL * hg, HL * (hg + 1))

        def tiles(x, permute):
            xs = x[b][:, hs, :].transpose(1, 0, 2)     # [HL, S, D]
            if permute:
                xs = xs[..., _ROPE_PERM]
            # [NJ, NT, 128, D] -> [NJ, 128, NT, D] partition-major
            return np.ascontiguousarray(
                xs.reshape(NJ, NT, 128, D).transpose(0, 2, 1, 3), np.float32)

        qk_c = np.concatenate([tiles(qh, True), tiles(kh, True)], axis=3)
        v_c = np.concatenate(
            [tiles(vh, False), np.ones((NJ, 128, NT, 1), np.float32)], axis=3)
        fs = slice(256 * hg, 256 * (hg + 1))
        wt_c = np.ascontiguousarray(
            proj_w[:, fs].T.reshape(2, 128, E).transpose(1, 0, 2), np.float32)
        maps.append({
            "qk": np.ascontiguousarray(qk_c),
            "v": np.ascontiguousarray(v_c),
            "cs": cs_t, "wt": wt_c,
        })
    return maps


_NC_CACHE = []


def _get_nc():
    if not _NC_CACHE:
        _NC_CACHE.append(build_nc())
    return _NC_CACHE[0]


def kernel(q, k, v, attn_mask, padding_mask, qn_w, kn_w, proj_w, proj_b,
           cos, sin):
    q = np.asarray(q, np.float32)
    k = np.asarray(k, np.float32)
    v = np.asarray(v, np.float32)
    proj_w = np.asarray(proj_w, np.float32)
    proj_b = np.asarray(proj_b, np.float32)
    cos = np.asarray(cos, np.float32)
    sin = np.asarray(sin, np.float32)
    attn_mask = np.asarray(attn_mask)
    padding_mask = np.asarray(padding_mask)
    qn_w = np.asarray(qn_w, np.float32)
    kn_w = np.asarray(kn_w, np.float32)
    # The kernel bakes in: causal attn_mask, no padding, unit RMSNorm weights.
    assert np.array_equal(
        attn_mask.reshape(S, S), np.tril(np.ones((S, S), attn_mask.dtype)))
    assert padding_mask.all()
    assert np.all(qn_w == 1.0) and np.all(kn_w == 1.0)

    in_maps = _shard_inputs(q, k, v, cos, sin, proj_w)
    nc = _get_nc()
    res = run_bass_kernel_spmd(nc, in_maps, core_ids=list(range(N_CORES)))
    parts = np.stack([r["out"].reshape(S, E) for r in res.results])
    full = np.empty((B, S, E), np.float32)
    for b in range(B):
        full[b] = parts[4 * b:4 * b + 4].astype(np.float32).sum(axis=0)
    return (full + proj_b[None, None, :]).astype(np.float32)
